# revision 1
# baseline (speedup 1.0000x reference)
"""Trainium2 Bass kernel for the LSTM seq2seq autoencoder.

Strategy:
  - Data-parallel over batch: B=512 -> 64 rows per core on 8 cores.
  - Layout A on-chip: batch on partitions (64), features on free dim.
  - All transposes of the *data* (input, output) are done on HOST numpy:
    device streams x^T tiles and emits y^T tiles.
  - Encoder length masking:
      c is frozen exactly by forcing gate preactivations (i -> -BIG,
      f -> +BIG) through an extra (mbar_t x FREEZE) rank-1 matmul row
      packed into the per-step lhsT; h is frozen with a 3-op masked blend.
  - Decoder feedback y_{t-1} @ Wih.T is algebraically folded into the
    recurrence: W_comb = Whh + Wih_dec @ out_W, so the autoregressive
    chain is a single K=256 matmul per step; y itself is computed off the
    critical path purely for output.
  - Gate order permuted to [i, f, o, g] so one sigmoid covers i,f,o.
"""

import numpy as np
from contextlib import ExitStack

import concourse.bass as bass
import concourse.bacc as bacc
import concourse.mybir as mybir
import concourse.tile as tile
from concourse.tile import add_dep_helper
from concourse.bass_utils import run_bass_kernel_spmd

B, T, D, H = 512, 512, 64, 256
G4 = 4 * H  # 1024
NCORES = 8
BL = B // NCORES  # 64
TDEC = T - 1      # 511 decoder steps
BIG = 30000.0
F32 = mybir.dt.float32
F32R = mybir.dt.float32r
BF16 = mybir.dt.bfloat16

_PROGRAM = None


def _gate_perm():
    # torch gate order i,f,g,o -> ours i,f,g,o (identity; bank0=[i,f], bank1=[g,o])
    r = np.arange(H)
    return np.concatenate([r, H + r, 2 * H + r, 3 * H + r])


def build_program(t_enc=T, t_dec=TDEC):
    nc = bacc.Bacc(None, target_bir_lowering=False)
    f = F32
    xp_d = nc.dram_tensor("xp", [t_enc, 66, BL], F32R, kind="ExternalInput")
    x0p_d = nc.dram_tensor("x0p", [65, BL], F32R, kind="ExternalInput")
    wxenc_d = nc.dram_tensor("wxenc", [66, G4], F32R, kind="ExternalInput")
    whhenc_d = nc.dram_tensor("whhenc", [128, 2, G4], F32R, kind="ExternalInput")
    whhdec_d = nc.dram_tensor("whhdec", [128, 2, G4], F32R, kind="ExternalInput")
    wcomb_d = nc.dram_tensor("wcomb", [128, 2, G4], F32R, kind="ExternalInput")
    wxdec_d = nc.dram_tensor("wxdec", [65, G4], F32R, kind="ExternalInput")
    bcomb_d = nc.dram_tensor("bcomb", [1, G4], F32R, kind="ExternalInput")
    outw_d = nc.dram_tensor("outw", [128, 2, D], F32R, kind="ExternalInput")
    outb_d = nc.dram_tensor("outb", [1, D], f, kind="ExternalInput")
    outbc_d = nc.dram_tensor("outbc", [D, 1], f, kind="ExternalInput")
    masks_d = nc.dram_tensor("masks", [BL, 2, t_enc], f, kind="ExternalInput")
    ident_d = nc.dram_tensor("ident", [64, 64], f, kind="ExternalInput")
    yt_d = nc.dram_tensor("yt", [t_dec + 1, D, BL], f, kind="ExternalOutput")

    Sig = mybir.ActivationFunctionType.Sigmoid
    Tanh = mybir.ActivationFunctionType.Tanh

    with ExitStack() as ctx:
        tc = ctx.enter_context(tile.TileContext(nc))
        singles = ctx.enter_context(tc.tile_pool(name="singles", bufs=1))
        xpool = ctx.enter_context(tc.tile_pool(name="xpool", bufs=6))
        work = ctx.enter_context(tc.tile_pool(name="work", bufs=3))
        hpool = ctx.enter_context(tc.tile_pool(name="hpool", bufs=2))
        cpool = ctx.enter_context(tc.tile_pool(name="cpool", bufs=2))
        htp = ctx.enter_context(tc.tile_pool(name="htp", bufs=2))
        oap = ctx.enter_context(tc.tile_pool(name="oap", bufs=2))
        gpool = ctx.enter_context(
            tc.tile_pool(name="gpool", bufs=2, space=bass.MemorySpace.PSUM))
        tpp = ctx.enter_context(
            tc.tile_pool(name="tpp", bufs=1, space=bass.MemorySpace.PSUM))
        ypool = ctx.enter_context(
            tc.tile_pool(name="ypool", bufs=2, space=bass.MemorySpace.PSUM))

        # ---- persistent constants ----
        s_wxenc = singles.tile([66, G4], F32R)
        nc.sync.dma_start(s_wxenc, wxenc_d[:, :])
        s_whhenc = singles.tile([128, 2, G4], F32R)
        nc.sync.dma_start(s_whhenc, whhenc_d[:, :, :])
        s_whhdec = singles.tile([128, 2, G4], F32R)
        nc.sync.dma_start(s_whhdec, whhdec_d[:, :, :])
        s_wcomb = singles.tile([128, 2, G4], F32R)
        nc.sync.dma_start(s_wcomb, wcomb_d[:, :, :])
        s_wxdec = singles.tile([65, G4], F32R)
        nc.sync.dma_start(s_wxdec, wxdec_d[:, :])
        s_bcomb = singles.tile([1, G4], F32R)
        nc.sync.dma_start(s_bcomb, bcomb_d[:, :])
        s_outw = singles.tile([128, 2, D], F32R)
        nc.sync.dma_start(s_outw, outw_d[:, :, :])
        s_outb = singles.tile([1, D], f)
        nc.sync.dma_start(s_outb, outb_d[:, :])
        s_masks = singles.tile([BL, 2, t_enc], f)
        nc.sync.dma_start(s_masks, masks_d[:, :, :])
        s_ident0 = singles.tile([64, 64], f)
        nc.sync.dma_start(s_ident0, ident_d[:, :])
        s_x0p0 = singles.tile([65, BL], F32R)
        nc.sync.dma_start(s_x0p0, x0p_d[:, :])
        s_outbc = singles.tile([D, 1], f)
        nc.sync.dma_start(s_outbc, outbc_d[:, :])
        s_ones0 = singles.tile([1, BL], f)
        nc.vector.memset(s_ones0, 1.0)
        s_ones = singles.tile([1, BL], F32R, tag="onesr")
        nc.vector.tensor_copy(s_ones, s_ones0)
        # route first-touch deps of matmul operands through DVE (one sem)
        s_ident = singles.tile([64, 64], f, tag="identv")
        nc.vector.tensor_copy(s_ident, s_ident0)
        s_identb = singles.tile([64, 64], BF16, tag="identb")
        nc.vector.tensor_copy(s_identb, s_ident0)
        s_x0p = singles.tile([65, BL], F32R, tag="x0pv")
        nc.vector.tensor_copy(s_x0p, s_x0p0)
        s_bcomb0 = s_bcomb
        s_bcomb = singles.tile([1, G4], F32R, tag="bcombv")
        nc.vector.tensor_copy(s_bcomb, s_bcomb0)
        s_outw0 = s_outw
        s_outw = singles.tile([128, 2, D], F32R, tag="outwv")
        nc.vector.tensor_copy(s_outw, s_outw0)

        # ---- initial state ----
        h_prev = singles.tile([BL, H], f, tag="h0")
        nc.vector.memset(h_prev, 0.0)
        c_prev = singles.tile([BL, H], f, tag="c0")
        nc.vector.memset(c_prev, 0.0)
        hT0f = singles.tile([128, 2, BL], f, tag="ht0f")
        nc.vector.memset(hT0f, 0.0)
        hT_init = singles.tile([128, 2, BL], F32R, tag="ht0")
        nc.vector.tensor_copy(hT_init, hT0f)
        hT_prev = (hT_init[:, 0, :], hT_init[:, 1, :])
        o_acc = singles.tile([BL, H], f, tag="oacc0")
        nc.vector.memset(o_acc, 0.0)

        def chain_order(*insts):
            for a, b in zip(insts[1:], insts[:-1]):
                add_dep_helper(a.ins, b.ins, sync=False, reason="order")

        def open_banks(lhs, rhs):
            """Allocate a step's gate psum banks; write the x/bias part."""
            ps0 = gpool.tile([BL, 512], f, tag="g0")
            ps1 = gpool.tile([BL, 512], f, tag="g1")
            nc.tensor.matmul(ps0, lhs, rhs[:, 0:512], start=True, stop=False)
            nc.tensor.matmul(ps1, lhs, rhs[:, 512:1024], start=True, stop=False)
            return (ps0, ps1)

        def h_matmuls(psb, whh):
            b0_last = None
            for nb in range(2):
                sl = slice(nb * 512, (nb + 1) * 512)
                m1 = nc.tensor.matmul(psb[nb], hT_prev[0],
                                      whh[:, 0, sl], start=False, stop=False)
                m2 = nc.tensor.matmul(psb[nb], hT_prev[1],
                                      whh[:, 1, sl], start=False, stop=True)
                if nb == 0:
                    b0_last = m2
                else:
                    add_dep_helper(m1.ins, b0_last.ins, sync=False,
                                   reason="bank0 first")
                    add_dep_helper(m2.ins, b0_last.ins, sync=False,
                                   reason="bank0 first")
            return m2

        def cell_mid(ps0, ps1, masked_t):
            """gates -> (o_t, tc_t); updates c_prev/o_acc.
            ig/c2/tanh_c half-split so half 0 races to the transpose."""
            nonlocal c_prev, o_acc
            HH = H // 2
            s_if = work.tile([BL, 2 * H], BF16, tag="sif")
            nc.scalar.activation(s_if, ps0, Sig)
            g_t = work.tile([BL, H], BF16, tag="gt")
            nc.scalar.activation(g_t, ps1[:, 0:H], Tanh)
            o_t = work.tile([BL, H], BF16, tag="ot")
            nc.scalar.activation(o_t, ps1[:, H:2 * H], Sig)
            fc = work.tile([BL, H], f, tag="fc")
            fci = nc.vector.tensor_mul(fc, s_if[:, H:2 * H], c_prev)
            ig = work.tile([BL, H], BF16, tag="ig")
            c_new = cpool.tile([BL, H], f, tag="c")
            tc_t = work.tile([BL, H], BF16, tag="tct")
            dchain = [fci]
            achain = []
            for hh in range(2):
                s = slice(hh * HH, (hh + 1) * HH)
                dchain.append(nc.vector.tensor_mul(ig[:, s], s_if[:, s],
                                                   g_t[:, s]))
                dchain.append(nc.vector.tensor_add(c_new[:, s], fc[:, s],
                                                   ig[:, s]))
                achain.append(nc.scalar.activation(tc_t[:, s],
                                                   c_new[:, s], Tanh))
            cell_mid.last_c_add = dchain[-1]
            chain_order(*dchain)
            chain_order(*achain)
            cell_mid.dve_tail = dchain[-1]
            cell_mid.mask_ops = None
            if masked_t is not None:
                # capture o at the freeze step: o_acc += o_t * e_t (off-chain)
                oam = work.tile([BL, H], f, tag="oam")
                om = nc.gpsimd.tensor_scalar_mul(
                    oam, o_t, s_masks[:, 0, masked_t:masked_t + 1])
                o_acc2 = oap.tile([BL, H], f, tag="oacc")
                oa = nc.gpsimd.tensor_add(o_acc2, o_acc, oam)
                chain_order(om, oa)
                cell_mid.mask_ops = (om, oa)
                o_acc = o_acc2
            c_prev = c_new
            return o_t, tc_t

        def tail_transpose(o_t, tc_t):
            """h2 = o*tanh(c) in halves; transpose+copy each half ASAP."""
            nonlocal h_prev, hT_prev
            h_new = hpool.tile([BL, H], BF16, tag="h")
            tp0 = tpp.tile([128, BL], BF16, tag="tp0")
            tp1 = tpp.tile([128, BL], BF16, tag="tp1")
            hT0 = htp.tile([128, BL], F32R, tag="hT0")
            hT1 = htp.tile([128, BL], F32R, tag="hT1")
            h20 = nc.vector.tensor_mul(h_new[:, 0:128], o_t[:, 0:128],
                                       tc_t[:, 0:128])
            nc.tensor.transpose(tp0, h_new[:, 0:128], s_identb)
            h21 = nc.vector.tensor_mul(h_new[:, 128:256], o_t[:, 128:256],
                                       tc_t[:, 128:256])
            nc.tensor.transpose(tp1, h_new[:, 128:256], s_identb)
            cp0 = nc.vector.tensor_copy(hT0, tp0)
            nc.scalar.copy(hT1, tp1)
            chain_order(cell_mid.dve_tail, h20, h21, cp0)

            h_prev = h_new
            hT_prev = (hT0, hT1)

        def transpose_full(h_new):
            nonlocal hT_prev
            tp0 = tpp.tile([128, BL], f, tag="tp0")
            tp1 = tpp.tile([128, BL], f, tag="tp1")
            hT0 = htp.tile([128, BL], F32R, tag="hT0")
            hT1 = htp.tile([128, BL], F32R, tag="hT1")
            nc.tensor.transpose(tp0, h_new[:, 0:128], s_ident)
            nc.tensor.transpose(tp1, h_new[:, 128:256], s_ident)
            nc.scalar.copy(hT0, tp0)
            nc.vector.tensor_copy(hT1, tp1)
            hT_prev = (hT0, hT1)

        # ================= ENCODER =================
        xp_t = xpool.tile([66, BL], F32R, tag="xp")
        nc.sync.dma_start(xp_t, xp_d[0, :, :])
        psb = open_banks(xp_t, s_wxenc)
        for t in range(t_enc):
            h_matmuls(psb, s_whhenc)
            o_t, tc_t = cell_mid(psb[0], psb[1], t)
            if t + 1 < t_enc:
                xp_t = xpool.tile([66, BL], F32R, tag="xp")
                nc.sync.dma_start(xp_t, xp_d[t + 1, :, :])
                psb = open_banks(xp_t, s_wxenc)
            tail_transpose(o_t, tc_t)

        # ===== boundary: h_enc = o_acc * tanh(c_final) =====
        psb = open_banks(s_x0p, s_wxdec)
        tc_e = work.tile([BL, H], f, tag="tct")
        nc.scalar.activation(tc_e, c_prev, Tanh)
        h_enc = hpool.tile([BL, H], f, tag="h")
        nc.vector.tensor_mul(h_enc, o_acc, tc_e)
        transpose_full(h_enc)

        # ================= DECODER =================
        pending_y = None
        for j in range(t_dec):
            whh = s_whhdec if j == 0 else s_wcomb
            last_h = h_matmuls(psb, whh)
            o_t, tc_t = cell_mid(psb[0], psb[1], None)
            if pending_y is not None:
                hTp, slot = pending_y
                yps = ypool.tile([D, BL], f, tag="y")
                for kc in range(2):
                    ym = nc.tensor.matmul(yps, s_outw[:, kc, :], hTp[kc],
                                          start=(kc == 0), stop=(kc == 1))
                    add_dep_helper(ym.ins, last_h.ins, sync=False,
                                   reason="y after h MMs")
                y_sb = work.tile([D, BL], f, tag="ysb")
                ysb_i = nc.vector.tensor_scalar_add(y_sb, yps, s_outbc)
                add_dep_helper(ysb_i.ins, cell_mid.last_c_add.ins, sync=False,
                               reason="y_sb after c2")
                nc.sync.dma_start(yt_d[slot, :, :], y_sb)
            if j + 1 < t_dec:
                psb = open_banks(s_ones, s_bcomb)
            tail_transpose(o_t, tc_t)
            pending_y = (hT_prev, j + 1)
        # final y
        hTp, slot = pending_y
        yps = ypool.tile([D, BL], f, tag="y")
        for kc in range(2):
            nc.tensor.matmul(yps, s_outw[:, kc, :], hTp[kc],
                             start=(kc == 0), stop=(kc == 1))
        y_sb = work.tile([D, BL], f, tag="ysb")
        nc.vector.tensor_scalar_add(y_sb, yps, s_outbc)
        nc.sync.dma_start(yt_d[slot, :, :], y_sb)

    nc.compile()
    return nc


def _prep_host(inputs, t_enc=T, t_dec=TDEC):
    """Build per-core in_maps from full inputs (numpy, all fp32)."""
    perm = _gate_perm()
    x = np.asarray(inputs["input_tensor"], np.float32)
    tgt = np.asarray(inputs["target_tensor"], np.float32)
    lens = np.asarray(inputs["lens"]).astype(np.int64)

    eWih = np.asarray(inputs["enc_Wih"], np.float32)[perm]
    eWhh = np.asarray(inputs["enc_Whh"], np.float32)[perm]
    eb = (np.asarray(inputs["enc_bih"], np.float32)
          + np.asarray(inputs["enc_bhh"], np.float32))[perm]
    dWih = np.asarray(inputs["dec_Wih"], np.float32)[perm]
    dWhh = np.asarray(inputs["dec_Whh"], np.float32)[perm]
    db = (np.asarray(inputs["dec_bih"], np.float32)
          + np.asarray(inputs["dec_bhh"], np.float32))[perm]
    oW = np.asarray(inputs["out_W"], np.float32)
    ob = np.asarray(inputs["out_b"], np.float32)

    freeze = np.zeros(G4, np.float32)
    freeze[0:H] = -BIG      # i -> 0
    freeze[H:2 * H] = BIG   # f -> 1

    wxenc = np.concatenate([eWih.T, eb[None, :], freeze[None, :]], 0)  # [66,G4]
    whhencT = eWhh.T.reshape(2, 128, G4).transpose(1, 0, 2).copy()     # [128,2,G4]
    whhdecT = dWhh.T.reshape(2, 128, G4).transpose(1, 0, 2).copy()
    wcomb = dWhh + dWih @ oW                                           # [G4,H]
    wcombT = wcomb.T.reshape(2, 128, G4).transpose(1, 0, 2).copy()
    bcomb = (db + dWih @ ob)[None, :]                                  # [1,G4]
    wxdec = np.concatenate([dWih.T, db[None, :]], 0)                   # [65,G4]
    outwT = oW.T.reshape(2, 128, D).transpose(1, 0, 2).copy()          # [128,2,D]
    outb = ob[None, :]
    ident = np.eye(64, dtype=np.float32)

    tt = np.arange(t_enc)[None, :]
    in_maps = []
    for c in range(NCORES):
        b0 = c * BL
        xs = x[b0:b0 + BL, :t_enc, :]                # [BL,t,D]
        xp = np.empty((t_enc, 66, BL), np.float32)
        xp[:, 0:D, :] = xs.transpose(1, 2, 0)
        xp[:, D, :] = 1.0
        lc = lens[b0:b0 + BL]
        mbar = (tt >= lc[:, None]).astype(np.float32)  # [BL,t]
        xp[:, D + 1, :] = mbar.T
        efreeze = (tt == (lc[:, None] - 1)).astype(np.float32)  # [BL,t]
        x0p = np.empty((65, BL), np.float32)
        x0p[0:D, :] = tgt[b0:b0 + BL, 0, :].T
        x0p[D, :] = 1.0
        masks = np.stack([efreeze, mbar], 1)           # [BL,2,t]
        in_maps.append({
            "xp": np.ascontiguousarray(xp),
            "x0p": x0p,
            "wxenc": wxenc, "whhenc": whhencT, "whhdec": whhdecT,
            "wcomb": wcombT, "wxdec": wxdec, "bcomb": bcomb,
            "outw": outwT, "outb": outb, "outbc": ob[:, None].copy(),
            "masks": np.ascontiguousarray(masks),
            "ident": ident,
        })
    return in_maps, lens


def kernel(**inputs) -> np.ndarray:
    global _PROGRAM
    if _PROGRAM is None:
        _PROGRAM = build_program()
    nc = _PROGRAM
    in_maps, lens = _prep_host(inputs)
    res = run_bass_kernel_spmd(nc, in_maps, core_ids=list(range(NCORES)))
    out = np.zeros((B, T, D), np.float32)
    for c in range(NCORES):
        yt = res.results[c]["yt"]                      # [T, D, BL]
        out[c * BL:(c + 1) * BL] = yt.transpose(2, 0, 1)
    mask = (np.arange(T)[None, :] < lens[:, None])[:, :, None]
    out *= mask
    out[:, 0, :] = 0.0
    return out



# revision 2
# speedup vs baseline: 1.0507x; 1.0507x over previous
"""Trainium2 Bass kernel for the LSTM seq2seq autoencoder (v2, packed layout).

Strategy (per core, 64 batch rows, 1023 serial LSTM steps):
  - Packed layout: every on-chip elementwise tensor is [128, 128] with
    partition p = b (rows 0:64, feature dims 0:128) or 64+b (dims 128:256).
    Gates live in TWO PSUM banks of [128, 256]: bank A = [f|g], bank B =
    [i|o], each gate a packed [128,128] tile.  All ACT/DVE ops use the full
    128 partitions; sigmoid(f) starts as soon as bank A completes.
  - All matmul operands bf16 (fp32 streams at half rate on PE).
  - One [128,128] PE transpose of packed h per step yields both K-halves
    of h^T side by side; one DVE copy to SBUF feeds the next step's MMs.
  - Encoder h_enc capture: h_acc(PSUM) += Hp_t^T @ E_t accumulated over all
    512 steps, where E_t = diag(e_t) (e_t[b]=1 iff t==len_b-1) streamed as
    bf16 input data.  c freezes via the (i->-BIG, f->+BIG) gate-preactivation
    trick driven by the mbar row of xp.  No GPSIMD anywhere.
  - Decoder feedback folded: W_comb = Whh + Wih_dec @ out_W (as v1).
  - y bias-add on ScalarE via per-partition bias AP.
"""

import numpy as np
import ml_dtypes
from contextlib import ExitStack

import concourse.bass as bass
import concourse.bacc as bacc
import concourse.mybir as mybir
import concourse.tile as tile
from concourse.tile import add_dep_helper
from concourse.bass_utils import run_bass_kernel_spmd

B, T, D, H = 512, 512, 64, 256
G4 = 4 * H
NCORES = 8
BL = B // NCORES          # 64
TDEC = T - 1              # 511 decoder steps
BIG = 30000.0
F32 = mybir.dt.float32
BF16 = mybir.dt.bfloat16
BF = ml_dtypes.bfloat16

_PROGRAM = None

# Weight column layout (1024 device gate columns):
#   [0:256)    bank A, top rows   : f dims 0:128   | i dims 0:128
#   [256:512)  bank A, bottom rows: f dims 128:256 | i dims 128:256
#   [512:768)  bank B, top rows   : g dims 0:128   | o dims 0:128
#   [768:1024) bank B, bottom rows: g dims 128:256 | o dims 128:256
# torch row order in the 4H weight matrices: i(0:H), f(H:2H), g(2H:3H), o(3H:4H)


def _colmap():
    r = np.arange(128)
    return np.concatenate([
        H + r, 0 + r,                  # A top: f0, i0
        H + 128 + r, 128 + r,          # A bot: f1, i1
        2 * H + r, 3 * H + r,          # B top: g0, o0
        2 * H + 128 + r, 3 * H + 128 + r,  # B bot: g1, o1
    ])


def build_program(t_enc=T, t_dec=TDEC):
    nc = bacc.Bacc(None, target_bir_lowering=False)
    NB = t_enc // 4  # xp/E stream blocks

    xp_d = nc.dram_tensor("xp", [NB, 66, 4, BL], BF16, kind="ExternalInput")
    e_d = nc.dram_tensor("ed", [NB, 128, 4, 128], BF16, kind="ExternalInput")
    x0p_d = nc.dram_tensor("x0p", [65, BL], BF16, kind="ExternalInput")
    wxenc_d = nc.dram_tensor("wxenc", [66, 1024], BF16, kind="ExternalInput")
    whhenc_d = nc.dram_tensor("whhenc", [128, 2, 1024], BF16, kind="ExternalInput")
    whhdec_d = nc.dram_tensor("whhdec", [128, 2, 1024], BF16, kind="ExternalInput")
    wcomb_d = nc.dram_tensor("wcomb", [128, 2, 1024], BF16, kind="ExternalInput")
    wxdec_d = nc.dram_tensor("wxdec", [65, 1024], BF16, kind="ExternalInput")
    bcomb_d = nc.dram_tensor("bcomb", [1, 1024], BF16, kind="ExternalInput")
    outw_d = nc.dram_tensor("outw", [128, 2, D], BF16, kind="ExternalInput")
    outbc_d = nc.dram_tensor("outbc", [D, 1], F32, kind="ExternalInput")
    ident_d = nc.dram_tensor("ident", [128, 128], BF16, kind="ExternalInput")
    yt_d = nc.dram_tensor("yt", [t_dec + 1, D, BL], F32, kind="ExternalOutput")

    Sig = mybir.ActivationFunctionType.Sigmoid
    Tanh = mybir.ActivationFunctionType.Tanh
    Ident = mybir.ActivationFunctionType.Identity

    with ExitStack() as ctx:
        tc = ctx.enter_context(tile.TileContext(nc))
        singles = ctx.enter_context(tc.tile_pool(name="singles", bufs=1))
        xpool = ctx.enter_context(tc.tile_pool(name="xpool", bufs=3))
        epool = ctx.enter_context(tc.tile_pool(name="epool", bufs=3))
        work = ctx.enter_context(tc.tile_pool(name="work", bufs=3))
        cpool = ctx.enter_context(tc.tile_pool(name="cpool", bufs=2))
        hpool = ctx.enter_context(tc.tile_pool(name="hpool", bufs=2))
        htp = ctx.enter_context(tc.tile_pool(name="htp", bufs=2))
        gpoolA = ctx.enter_context(
            tc.tile_pool(name="gpoolA", bufs=2, space=bass.MemorySpace.PSUM))
        gpoolB = ctx.enter_context(
            tc.tile_pool(name="gpoolB", bufs=2, space=bass.MemorySpace.PSUM))
        tpp = ctx.enter_context(
            tc.tile_pool(name="tpp", bufs=1, space=bass.MemorySpace.PSUM))
        ypool = ctx.enter_context(
            tc.tile_pool(name="ypool", bufs=1, space=bass.MemorySpace.PSUM))
        cbp = ctx.enter_context(
            tc.tile_pool(name="cbp", bufs=1, space=bass.MemorySpace.PSUM))
        accp = ctx.enter_context(
            tc.tile_pool(name="accp", bufs=1, space=bass.MemorySpace.PSUM))

        # ---- persistent constants ----
        s_wxenc = singles.tile([66, 1024], BF16)
        nc.sync.dma_start(s_wxenc, wxenc_d[:, :])
        s_whhenc = singles.tile([128, 2, 1024], BF16)
        nc.sync.dma_start(s_whhenc, whhenc_d[:, :, :])
        s_whhdec = singles.tile([128, 2, 1024], BF16)
        nc.sync.dma_start(s_whhdec, whhdec_d[:, :, :])
        s_wcomb = singles.tile([128, 2, 1024], BF16)
        nc.sync.dma_start(s_wcomb, wcomb_d[:, :, :])
        s_wxdec = singles.tile([65, 1024], BF16)
        nc.sync.dma_start(s_wxdec, wxdec_d[:, :])
        s_bcomb0 = singles.tile([1, 1024], BF16)
        nc.sync.dma_start(s_bcomb0, bcomb_d[:, :])
        s_outw0 = singles.tile([128, 2, D], BF16)
        nc.sync.dma_start(s_outw0, outw_d[:, :, :])
        s_outbc = singles.tile([D, 1], F32)
        nc.sync.dma_start(s_outbc, outbc_d[:, :])
        s_ident0 = singles.tile([128, 128], BF16)
        nc.sync.dma_start(s_ident0, ident_d[:, :])
        s_x0p0 = singles.tile([65, BL], BF16)
        nc.sync.dma_start(s_x0p0, x0p_d[:, :])

        s_ones0 = singles.tile([1, BL], F32)
        nc.vector.memset(s_ones0, 1.0)
        s_ones = singles.tile([1, BL], BF16, tag="onesb")
        nc.vector.tensor_copy(s_ones, s_ones0)
        # route first-touch deps of matmul operands through DVE (one sem)
        s_ident = singles.tile([128, 128], BF16, tag="identv")
        nc.vector.tensor_copy(s_ident, s_ident0)
        s_x0p = singles.tile([65, BL], BF16, tag="x0pv")
        nc.vector.tensor_copy(s_x0p, s_x0p0)
        s_bcomb = singles.tile([1, 1024], BF16, tag="bcombv")
        nc.vector.tensor_copy(s_bcomb, s_bcomb0)
        s_outw = singles.tile([128, 2, D], BF16, tag="outwv")
        nc.vector.tensor_copy(s_outw, s_outw0)

        # ---- initial state ----
        # c state double-buffered inside ONE PSUM bank (slices alternate)
        cbank = cbp.tile([128, 256], F32, tag="cb")
        nc.vector.memset(cbank[:, 0:128], 0.0)
        hT0f = singles.tile([128, 128], F32, tag="ht0f")
        nc.vector.memset(hT0f, 0.0)
        hT_init = singles.tile([128, 128], BF16, tag="ht0")
        nc.vector.tensor_copy(hT_init, hT0f)

        state = {"ci": 0, "hT": hT_init, "act": None, "dve": None,
                 "pe": None}

        def chain(kind, inst):
            prev = state[kind]
            if prev is not None:
                add_dep_helper(inst.ins, prev.ins, sync=False, reason=kind)
            state[kind] = inst
            return inst

        def mm(*args, **kwargs):
            return chain("pe", nc.tensor.matmul(*args, **kwargs))

        def h_matmuls(psA, psB, whh):
            """8 h-MMs of N=256: bank A (f,g) first, then bank B (i,o)."""
            hT = state["hT"]
            last = None
            for bank_i, ps in ((0, psA), (1, psB)):
                for k in range(2):          # K-halves of h
                    for tb in range(2):     # top rows / bottom rows
                        last = mm(
                            ps[tb * 64:(tb + 1) * 64, :],
                            hT[:, k * 64:(k + 1) * 64],
                            whh[:, k, bank_i * 512 + tb * 256:
                                bank_i * 512 + (tb + 1) * 256],
                            start=False, stop=(k == 1))
            return last

        def open_banks_x(lhs, rhs):
            """x/bias part: 4 MMs N=256, start=True."""
            psA = gpoolA.tile([128, 256], F32, tag="gA")
            psB = gpoolB.tile([128, 256], F32, tag="gB")
            for bank_i, ps in ((0, psA), (1, psB)):
                for tb in range(2):
                    mm(ps[tb * 64:(tb + 1) * 64, :],
                       lhs,
                       rhs[:, bank_i * 512 + tb * 256:
                           bank_i * 512 + (tb + 1) * 256],
                       start=True, stop=False)
            return psA, psB

        def cell_mid(psA, psB):
            """gates -> (o_t, tc_t); updates c slice in state."""
            fi_t = work.tile([128, 256], BF16, tag="fit")
            chain("act", nc.scalar.activation(fi_t, psA[:, 0:256], Sig))
            g_t = work.tile([128, 128], BF16, tag="gt")
            chain("act", nc.scalar.activation(g_t, psB[:, 0:128], Tanh))
            o_t = work.tile([128, 128], BF16, tag="ot")
            a3 = chain("act", nc.scalar.activation(o_t, psB[:, 128:256], Sig))

            ci = state["ci"]
            c_old = cbank[:, ci * 128:(ci + 1) * 128]
            c_new = cbank[:, (1 - ci) * 128:(2 - ci) * 128]
            fc = work.tile([128, 128], F32, tag="fc")
            chain("dve", nc.vector.tensor_mul(fc, fi_t[:, 0:128], c_old))
            ig = work.tile([128, 128], BF16, tag="ig")
            chain("dve", nc.vector.tensor_mul(ig, fi_t[:, 128:256], g_t))
            d3 = chain("dve", nc.vector.tensor_add(c_new, fc, ig))
            tc_t = work.tile([128, 128], BF16, tag="tct")
            a4 = chain("act", nc.scalar.activation(tc_t, c_new, Tanh))
            cell_mid.last_acts = (a4, d3)
            state["ci"] = 1 - ci
            return o_t, tc_t

        def tail_transpose(o_t, tc_t, tpB_pool):
            """h = o*tanh(c) -> two half transposes -> hT in SBUF.

            transpose(Hp[0:64,:]) = hT[:, 0:64] (K-half 0), so the K0
            matmuls can start after the first copy; the second half
            trails.  The two halves use different PSUM banks (PE-write +
            DVE-read same-bank collisions serialize)."""
            h_new = hpool.tile([128, 128], BF16, tag="h")
            chain("dve", nc.vector.tensor_mul(h_new, o_t, tc_t))
            tpA = tpp.tile([128, 64], BF16, tag="tp")
            chain("pe", nc.tensor.transpose(tpA, h_new[0:64, :],
                                            s_ident[0:64, 0:64]))
            tpB = tpB_pool.tile([128, 64], BF16,
                                tag="y" if tpB_pool is ypool else "hacc")
            chain("pe", nc.tensor.transpose(tpB, h_new[64:128, :],
                                            s_ident[64:128, 64:128]))
            hT = htp.tile([128, 128], BF16, tag="hT")
            chain("dve", nc.vector.tensor_copy(hT[:, 0:64], tpA))
            chain("dve", nc.vector.tensor_copy(hT[:, 64:128], tpB))
            state["hT"] = hT
            return h_new

        # ================= ENCODER =================
        h_acc = accp.tile([128, 128], F32, tag="hacc")
        xp_blk = xpool.tile([66, 4, BL], BF16, tag="xp")
        nc.sync.dma_start(xp_blk, xp_d[0, :, :, :])
        e_blk = epool.tile([128, 4, 128], BF16, tag="eb")
        nc.sync.dma_start(e_blk, e_d[0, :, :, :])
        psA, psB = open_banks_x(xp_blk[:, 0, :], s_wxenc)
        for t in range(t_enc):
            m, s = divmod(t, 4)
            h_matmuls(psA, psB, s_whhenc)
            io_t, tc_t = cell_mid(psA, psB)
            if s == 3 and m + 1 < NB:
                xp_blk2 = xpool.tile([66, 4, BL], BF16, tag="xp")
                nc.sync.dma_start(xp_blk2, xp_d[m + 1, :, :, :])
                e_blk2 = epool.tile([128, 4, 128], BF16, tag="eb")
                nc.sync.dma_start(e_blk2, e_d[m + 1, :, :, :])
            h_new = tail_transpose(io_t, tc_t, ypool)
            # capture: h_acc += Hp^T @ diag(e_t)
            mm(h_acc, h_new, e_blk[:, s, :],
               start=(t == 0), stop=(t == t_enc - 1))
            if s == 3 and m + 1 < NB:
                xp_blk, e_blk = xp_blk2, e_blk2
            if t + 1 < t_enc:
                psA, psB = open_banks_x(xp_blk[:, (t + 1) % 4, :], s_wxenc)

        # ===== boundary: hT_enc from h_acc; c_prev already frozen =====
        psA, psB = open_banks_x(s_x0p, s_wxdec)
        hT_enc = htp.tile([128, 128], BF16, tag="hT")
        chain("dve", nc.vector.tensor_copy(hT_enc, h_acc))
        state["hT"] = hT_enc

        # ================= DECODER =================
        pending_y = None

        def do_y(pending):
            hTp, slot = pending
            yps = ypool.tile([D, BL], F32, tag="y")
            for kc in range(2):
                mm(yps, s_outw[:, kc, :],
                   hTp[:, kc * 64:(kc + 1) * 64],
                   start=(kc == 0), stop=(kc == 1))
            y_sb = work.tile([D, BL], F32, tag="ysb")
            chain("dve", nc.vector.tensor_scalar_add(y_sb, yps, s_outbc))
            nc.sync.dma_start(yt_d[slot, :, :], y_sb)

        for j in range(t_dec):
            whh = s_whhdec if j == 0 else s_wcomb
            h_matmuls(psA, psB, whh)
            o_t, tc_t = cell_mid(psA, psB)
            if j + 1 < t_dec:
                psA, psB = open_banks_x(s_ones, s_bcomb)
            tail_transpose(o_t, tc_t, accp)
            if pending_y is not None:
                do_y(pending_y)
            pending_y = (state["hT"], j + 1)
        do_y(pending_y)

    nc.compile()
    return nc


def _prep_host(inputs, t_enc=T, t_dec=TDEC):
    """Build per-core in_maps from full inputs (numpy)."""
    cm = _colmap()
    x = np.asarray(inputs["input_tensor"], np.float32)
    tgt = np.asarray(inputs["target_tensor"], np.float32)
    lens = np.asarray(inputs["lens"]).astype(np.int64)

    eWih = np.asarray(inputs["enc_Wih"], np.float32)
    eWhh = np.asarray(inputs["enc_Whh"], np.float32)
    eb = (np.asarray(inputs["enc_bih"], np.float32)
          + np.asarray(inputs["enc_bhh"], np.float32))
    dWih = np.asarray(inputs["dec_Wih"], np.float32)
    dWhh = np.asarray(inputs["dec_Whh"], np.float32)
    db = (np.asarray(inputs["dec_bih"], np.float32)
          + np.asarray(inputs["dec_bhh"], np.float32))
    oW = np.asarray(inputs["out_W"], np.float32)
    ob = np.asarray(inputs["out_b"], np.float32)

    freeze = np.zeros(G4, np.float32)
    freeze[0:H] = -BIG      # i -> 0
    freeze[H:2 * H] = BIG   # f -> 1

    wxenc = np.concatenate([eWih.T, eb[None, :], freeze[None, :]], 0)[:, cm]
    wxdec = np.concatenate([dWih.T, db[None, :]], 0)[:, cm]       # [65, 1024]
    whhenc = eWhh.T[:, cm].reshape(2, 128, 1024).transpose(1, 0, 2).copy()
    whhdec = dWhh.T[:, cm].reshape(2, 128, 1024).transpose(1, 0, 2).copy()
    wcomb = (dWhh + dWih @ oW).T[:, cm].reshape(2, 128, 1024)\
        .transpose(1, 0, 2).copy()
    bcomb = (db + dWih @ ob)[cm][None, :]                         # [1, 1024]
    outw = oW.T.reshape(2, 128, D).transpose(1, 0, 2).copy()      # [128,2,D]
    ident = np.eye(128, dtype=np.float32)

    tt = np.arange(t_enc)[None, :]
    NB = t_enc // 4
    in_maps = []
    for c in range(NCORES):
        b0 = c * BL
        xs = x[b0:b0 + BL, :t_enc, :]                  # [BL,t,D]
        lc = lens[b0:b0 + BL]
        mbar = (tt >= lc[:, None]).astype(np.float32)  # [BL,t]
        xp = np.empty((t_enc, 66, BL), np.float32)
        xp[:, 0:D, :] = xs.transpose(1, 2, 0)
        xp[:, D, :] = 1.0
        xp[:, D + 1, :] = mbar.T
        xp = xp.reshape(NB, 4, 66, BL).transpose(0, 2, 1, 3)      # [NB,66,4,BL]

        efree = (tt == (lc[:, None] - 1)).astype(np.float32)      # [BL,t]
        ep = np.zeros((t_enc, 128, 128), np.float32)
        idx = np.arange(128)
        ep[:, idx, idx] = np.concatenate([efree.T, efree.T], 1)   # [t,128] diag
        ep = ep.reshape(NB, 4, 128, 128).transpose(0, 2, 1, 3)    # [NB,128,4,128]

        x0p = np.empty((65, BL), np.float32)
        x0p[0:D, :] = tgt[b0:b0 + BL, 0, :].T
        x0p[D, :] = 1.0
        in_maps.append({
            "xp": np.ascontiguousarray(xp).astype(BF),
            "ed": np.ascontiguousarray(ep).astype(BF),
            "x0p": x0p.astype(BF),
            "wxenc": wxenc.astype(BF), "whhenc": whhenc.astype(BF),
            "whhdec": whhdec.astype(BF), "wcomb": wcomb.astype(BF),
            "wxdec": wxdec.astype(BF), "bcomb": bcomb.astype(BF),
            "outw": outw.astype(BF), "outbc": ob[:, None].astype(np.float32),
            "ident": ident.astype(BF),
        })
    return in_maps, lens


def kernel(**inputs) -> np.ndarray:
    global _PROGRAM
    if _PROGRAM is None:
        _PROGRAM = build_program()
    nc = _PROGRAM
    in_maps, lens = _prep_host(inputs)
    res = run_bass_kernel_spmd(nc, in_maps, core_ids=list(range(NCORES)))
    out = np.zeros((B, T, D), np.float32)
    for c in range(NCORES):
        yt = res.results[c]["yt"]                      # [T, D, BL]
        out[c * BL:(c + 1) * BL] = yt.transpose(2, 0, 1)
    mask = (np.arange(T)[None, :] < lens[:, None])[:, :, None]
    out *= mask
    out[:, 0, :] = 0.0
    return out
